# revision 4
# baseline (speedup 1.0000x reference)
"""GCNConv (message passing + linear) on 8 Trainium2 NeuronCores.

Strategy (graph/data parallel, per sharding hint):
  - Source feature table = x itself in node-id order, cast bf16, split
    into two DRAM tables (A: ids < 32768, B: rest) to satisfy the int16
    gather-index range. No on-device prestage pass.
  - Destination nodes sorted by degree and dealt in blocks of 8*128
    across the 8 cores; PSUM accumulates pairs of 128-dst groups
    ([128 feat x 256 dst] tiles).
  - Each core bulk row-gathers per-edge messages with the Q7 dma_gather
    instruction (32 tiles = 4096 rows per call, amortizing the Q7
    per-call overhead) directly from the bf16 x tables. Message tiles
    are packed densely: a tile is any 128 edges of the pair, each
    carrying its destination slot pv and normalization scale s.
  - Segment-sum on the TensorEngine: msg tile [128 edge, 128 feat]
    (stationary) x one-hot rhs[k, n] = (n == pv[k]) * s[k] (built per
    tile on the Vector engine from host-provided integer slot ids and
    degree products; s = rsqrt(deg_src*deg_dst) computed on device).
  - Self-loops use a sequential (non-gather) DMA of owned rows plus one
    matmul per group with rhs scaled by 1/deg.
  - Final linear via W^T matmul + bias; output [d_out, local_dst];
    host unpermutes/transposes back to [N, d_out].

The Bass program is rebuilt per distinct edge_index (layout constants
are baked into the instruction stream); all 8 cores share one program
and differ only in their input data.
"""

import numpy as np

try:
    import ml_dtypes

    _BF16 = ml_dtypes.bfloat16
except ImportError:  # pragma: no cover
    _BF16 = None

import concourse.bacc as bacc
import concourse.bass as bass
import concourse.mybir as mybir
import concourse.tile as tile
from concourse.bass_utils import run_bass_kernel_spmd
from concourse.library_config import mlp as _mlp_lib
from concourse.tile_rust import add_dep_helper

P = 128
N_CORES = 8
TILES_PER_CALL = 8  # gather granularity; 1024 idxs = max per dma_gather call
SPLIT_MAX = 32768  # int16 gather index range per table
PAD_PV = 384.0  # out-of-range slot id: pad edges hit no psum column


def _wrap_idx16(linear_idx):
    """[n] int -> [128, n/16] int16 in the 16-partition wrapped, 8x
    replicated layout dma_gather expects (slot i at [i%16, i//16])."""
    n = linear_idx.shape[0]
    assert n % 16 == 0
    w = linear_idx.reshape(-1, 16).T.astype(np.int16)  # [16, n/16]
    return np.tile(w, (8, 1))


def _ragged_take(st, cnt):
    """Positions [st_i, st_i + cnt_i) concatenated, plus repeat index."""
    tot = int(cnt.sum())
    if tot == 0:
        return np.zeros(0, np.int64), np.zeros(0, np.int64)
    rep = np.repeat(np.arange(cnt.shape[0]), cnt)
    cum = np.cumsum(cnt) - cnt
    ofs = np.arange(tot) - np.repeat(cum, cnt)
    return np.repeat(st, cnt) + ofs, rep


# ----------------------------------------------------------------------------
# Host-side layout construction (sharding / index relabeling; degree
# products are exact integers — the rsqrt normalization happens on device).
# ----------------------------------------------------------------------------
def _prep(x, edge_index, weight, bias, n_cores):
    N, D = x.shape
    assert D == P
    src = np.asarray(edge_index[0], dtype=np.int64)
    dst = np.asarray(edge_index[1], dtype=np.int64)
    E = src.shape[0]

    deg = np.bincount(dst, minlength=N)
    count = (deg + 1).astype(np.int64)  # self-loop included

    SPLIT = min(SPLIT_MAX, N)
    NB_real = N - SPLIT

    in_A_src = src < SPLIT
    cntA = np.bincount(dst[in_A_src], minlength=N).astype(np.int64)
    cntB = deg - cntA

    order = np.argsort(count, kind="stable")
    BLK = n_cores * P
    GROUPS = (N + BLK - 1) // BLK
    PAIRS = (GROUPS + 1) // 2
    LOCAL_PAD = GROUPS * P
    OUTW = PAIRS * 2 * P

    # edges grouped per dst node id, A-sources first within each dst
    eorder = np.lexsort(((~in_A_src).astype(np.int8), dst))
    esrc = src[eorder]
    starts = np.zeros(N + 1, np.int64)
    starts[1:] = np.cumsum(deg)

    # per (core, pair) ragged message lists -> shared tile counts
    prange = np.arange(P)
    core_pair = []  # [core][pair] = (srcsA, slotsA, prodA, srcsB, slotsB, prodB)
    for c in range(n_cores):
        rows = []
        for p in range(PAIRS):
            sA, vA, pA = [], [], []
            sB, vB, pB = [], [], []
            for gg in range(2):
                g = 2 * p + gg
                if g >= GROUPS:
                    continue
                s_rank = BLK * g + n_cores * prange + c
                valid = s_rank < N
                nd = order[np.minimum(s_rank, N - 1)]
                ca = np.where(valid, cntA[nd], 0)
                cb = np.where(valid, cntB[nd], 0)
                st = starts[nd]
                cn = count[nd]
                posA, repA = _ragged_take(st, ca)
                srcsA = esrc[posA]
                sA.append(srcsA)
                vA.append(gg * P + repA)
                pA.append(count[srcsA] * cn[repA])
                posB, repB = _ragged_take(st + ca, cb)
                srcsB = esrc[posB] - SPLIT
                sB.append(srcsB)
                vB.append(gg * P + repB)
                pB.append(count[srcsB + SPLIT] * cn[repB])
            cat = lambda l: (
                np.concatenate(l) if l else np.zeros(0, np.int64)
            )
            core_pair.append(
                (cat(sA), cat(vA), cat(pA), cat(sB), cat(vB), cat(pB))
            )
    # core_pair index: c * PAIRS + p

    TgA, TgB = [], []
    for p in range(PAIRS):
        mA = max(
            (core_pair[c * PAIRS + p][0].shape[0] + P - 1) // P
            for c in range(n_cores)
        )
        mB = max(
            (core_pair[c * PAIRS + p][3].shape[0] + P - 1) // P
            for c in range(n_cores)
        )
        TgA.append(int(mA))
        TgB.append(int(mB))
    toffsA = np.zeros(PAIRS + 1, np.int64)
    toffsA[1:] = np.cumsum(TgA)
    toffsB = np.zeros(PAIRS + 1, np.int64)
    toffsB[1:] = np.cumsum(TgB)
    T_totalA = int(toffsA[-1])
    T_totalB = int(toffsB[-1])
    T_total = T_totalA + T_totalB

    xf = np.asarray(x, dtype=np.float32)
    xA = np.ascontiguousarray(xf[:SPLIT]).astype(_BF16)
    xB = (
        np.ascontiguousarray(xf[SPLIT:N]).astype(_BF16)
        if NB_real > 0
        else np.zeros((P, P), _BF16)
    )

    idxA_cores = np.zeros((n_cores, P, 8 * max(T_totalA, 1)), np.int16)
    idxB_cores = np.zeros((n_cores, P, 8 * max(T_totalB, 1)), np.int16)
    prod_cores = np.ones((n_cores, P, max(T_total, 1)), np.float32)
    pv_cores = np.full((n_cores, P, max(T_total, 1)), PAD_PV, np.float32)
    cself_cores = np.ones((n_cores, P, GROUPS), np.float32)
    pvself = np.empty((P, GROUPS), np.float32)
    for g in range(GROUPS):
        pvself[:, g] = (g % 2) * P + prange
    x_own = np.zeros((n_cores, LOCAL_PAD, P), _BF16)

    for c in range(n_cores):
        linA = np.zeros(max(T_totalA, 1) * P, np.int64)
        linB = np.zeros(max(T_totalB, 1) * P, np.int64)
        for p in range(PAIRS):
            sA, vA, pA, sB, vB, pB = core_pair[c * PAIRS + p]
            tg0 = int(toffsA[p] + toffsB[p])
            for (ss, vv, pp, lin, toffs, Tg, joff) in (
                (sA, vA, pA, linA, toffsA, TgA[p], 0),
                (sB, vB, pB, linB, toffsB, TgB[p], TgA[p]),
            ):
                n = ss.shape[0]
                if Tg == 0:
                    continue
                base = int(toffs[p]) * P
                lin[base : base + n] = ss
                # pv/prod arrays: [slot, col] with col = tg0+joff+j,
                # linear msg i -> tile j=i//P, slot k=i%P
                j_idx = np.arange(n) // P
                k_idx = np.arange(n) % P
                pv_cores[c][k_idx, tg0 + joff + j_idx] = vv
                prod_cores[c][k_idx, tg0 + joff + j_idx] = pp
        assert linA.min() >= 0 and linA.max() < max(SPLIT, 1)
        idxA_cores[c] = _wrap_idx16(linA)
        if T_totalB:
            assert linB.min() >= 0 and (NB_real == 0 or linB.max() < NB_real)
            idxB_cores[c] = _wrap_idx16(linB)

        ks = np.arange(LOCAL_PAD)
        s_rank = BLK * (ks // P) + n_cores * (ks % P) + c
        m = s_rank < N
        x_own[c][ks[m]] = xf[order[s_rank[m]]]
        cs = np.ones(LOCAL_PAD, np.float32)
        cs[ks[m]] = count[order[s_rank[m]]]
        cself_cores[c] = cs.reshape(GROUPS, P).T

    iota = np.tile(np.arange(2 * P, dtype=np.float32), (P, 1))
    wT = np.ascontiguousarray(np.asarray(weight, dtype=np.float32).T)
    bias_col = np.asarray(bias, dtype=np.float32).reshape(P, 1)

    return dict(
        N=N,
        D=D,
        E=E,
        n_cores=n_cores,
        SPLIT=SPLIT,
        NB_real=NB_real,
        GROUPS=GROUPS,
        PAIRS=PAIRS,
        LOCAL_PAD=LOCAL_PAD,
        OUTW=OUTW,
        TgA=TgA,
        TgB=TgB,
        toffsA=toffsA,
        toffsB=toffsB,
        T_totalA=T_totalA,
        T_totalB=T_totalB,
        T_total=T_total,
        xA=xA,
        xB=xB,
        x_own=x_own,
        prod_cores=prod_cores,
        pv_cores=pv_cores,
        cself_cores=cself_cores,
        pvself=pvself,
        iota=iota,
        idxA_cores=idxA_cores,
        idxB_cores=idxB_cores,
        wT=wT,
        bias_col=bias_col,
        order=order,
    )


# ----------------------------------------------------------------------------
# Device program
# ----------------------------------------------------------------------------
def _build(L):
    GROUPS = L["GROUPS"]
    PAIRS = L["PAIRS"]
    TgA, TgB = L["TgA"], L["TgB"]
    toffsA, toffsB = L["toffsA"], L["toffsB"]
    T_totalA, T_totalB = L["T_totalA"], L["T_totalB"]
    T_total = L["T_total"]
    LOCAL_PAD = L["LOCAL_PAD"]
    OUTW = L["OUTW"]
    NAr = L["SPLIT"]
    NBr = max(L["NB_real"], P)
    f32 = mybir.dt.float32
    bf16 = mybir.dt.bfloat16
    i16 = mybir.dt.int16
    AF = mybir.ActivationFunctionType
    W2 = 2 * P

    nc = bacc.Bacc("TRN2", debug=False, num_devices=L["n_cores"], num_swdge_queues=4)
    xA_dram = nc.dram_tensor("xA", [NAr, P], bf16, kind="ExternalInput")
    xB_dram = nc.dram_tensor("xB", [NBr, P], bf16, kind="ExternalInput")
    idxA_dram = nc.dram_tensor(
        "idxA", [P, 8 * max(T_totalA, 1)], i16, kind="ExternalInput"
    )
    idxB_dram = nc.dram_tensor(
        "idxB", [P, 8 * max(T_totalB, 1)], i16, kind="ExternalInput"
    )
    prod_dram = nc.dram_tensor("prod", [P, max(T_total, 1)], f32, kind="ExternalInput")
    pv_dram = nc.dram_tensor("pv", [P, max(T_total, 1)], f32, kind="ExternalInput")
    cself_dram = nc.dram_tensor("cself", [P, GROUPS], f32, kind="ExternalInput")
    pvself_dram = nc.dram_tensor("pvself", [P, GROUPS], f32, kind="ExternalInput")
    iota_dram = nc.dram_tensor("iota", [P, W2], f32, kind="ExternalInput")
    xown_dram = nc.dram_tensor("x_own", [LOCAL_PAD, P], bf16, kind="ExternalInput")
    wT_dram = nc.dram_tensor("wT", [P, P], f32, kind="ExternalInput")
    bias_dram = nc.dram_tensor("bias_col", [P, 1], f32, kind="ExternalInput")
    out_dram = nc.dram_tensor("out", [P, OUTW], f32, kind="ExternalOutput")

    with tile.TileContext(nc) as tc:
        with (
            tc.tile_pool(name="const", bufs=1) as cpool,
            tc.tile_pool(name="msg", bufs=7) as mpool,
            tc.tile_pool(name="rhs", bufs=24) as gpool,
            tc.tile_pool(name="xo", bufs=6) as xopool,
            tc.tile_pool(name="agg", bufs=4) as apool,
            tc.tile_pool(name="outs", bufs=2) as opool,
            tc.tile_pool(name="ps", bufs=4, space="PSUM") as pspool,
            tc.tile_pool(name="ps2", bufs=2, space="PSUM") as ps2pool,
        ):
            lib_inst = nc.gpsimd.load_library(_mlp_lib)

            # ---- constant loads (idx first: gathers depend on them)
            idxA_sb = cpool.tile([P, 8 * max(T_totalA, 1)], i16)
            nc.sync.dma_start(out=idxA_sb[:], in_=idxA_dram[:])
            idxB_sb = cpool.tile([P, 8 * max(T_totalB, 1)], i16)
            nc.sync.dma_start(out=idxB_sb[:], in_=idxB_dram[:])
            prod_sb = cpool.tile([P, max(T_total, 1)], f32)
            nc.sync.dma_start(out=prod_sb[:], in_=prod_dram[:])
            pv_sb = cpool.tile([P, max(T_total, 1)], f32)
            nc.sync.dma_start(out=pv_sb[:], in_=pv_dram[:])
            cself_sb = cpool.tile([P, GROUPS], f32)
            nc.sync.dma_start(out=cself_sb[:], in_=cself_dram[:])
            pvself_sb = cpool.tile([P, GROUPS], f32)
            nc.sync.dma_start(out=pvself_sb[:], in_=pvself_dram[:])
            iota_sb = cpool.tile([P, W2], f32)
            nc.sync.dma_start(out=iota_sb[:], in_=iota_dram[:])
            wT_sb = cpool.tile([P, P], f32)
            nc.sync.dma_start(out=wT_sb[:], in_=wT_dram[:])
            bias_sb = cpool.tile([P, 1], f32)
            nc.sync.dma_start(out=bias_sb[:], in_=bias_dram[:])

            # ---- normalization scales: s = rsqrt(count_src*count_dst)
            s_sb = cpool.tile([P, max(T_total, 1)], f32)
            nc.scalar.sqrt(s_sb[:], prod_sb[:])
            nc.vector.reciprocal(s_sb[:], s_sb[:])
            sinv_sb = cpool.tile([P, GROUPS], f32)
            nc.vector.reciprocal(sinv_sb[:], cself_sb[:])

            # ---- gather calls (lazy, in consumption order)
            msg_tiles = {}
            qrr = [0]

            def ensure_call(pass_key, k):
                key = (pass_key, k)
                if key in msg_tiles:
                    return
                T_tot = T_totalA if pass_key == "A" else T_totalB
                u_src = xA_dram if pass_key == "A" else xB_dram
                idx_sb = idxA_sb if pass_key == "A" else idxB_sb
                t0 = k * TILES_PER_CALL
                cnt = min(TILES_PER_CALL, T_tot - t0)
                m = mpool.tile([P, TILES_PER_CALL, P], bf16)
                g_inst = nc.gpsimd.dma_gather(
                    m[:, :cnt, :],
                    u_src[:, :],
                    idx_sb[:, 8 * t0 : 8 * (t0 + cnt)],
                    cnt * P,
                    cnt * P,
                    P,
                    queue_num=qrr[0] % 4,
                )
                qrr[0] += 1
                add_dep_helper(g_inst.ins, lib_inst.ins, reason="ucode lib before gather")
                msg_tiles[key] = m

            # ---- per pair of dst groups: segment-sum on PE + linear + bias
            out_t = None
            ostart = 0
            for p in range(PAIRS):
                psum = pspool.tile([P, W2], f32)
                tg0 = int(toffsA[p] + toffsB[p])
                j = 0
                for pass_key, Tp, toffs in (
                    ("A", TgA[p], toffsA),
                    ("B", TgB[p], toffsB),
                ):
                    for jj in range(Tp):
                        t = int(toffs[p]) + jj
                        k, kk = divmod(t, TILES_PER_CALL)
                        ensure_call(pass_key, k)
                        rhs = gpool.tile([P, W2], bf16)
                        nc.vector.tensor_scalar(
                            out=rhs[:],
                            in0=iota_sb[:],
                            scalar1=pv_sb[:, tg0 + j : tg0 + j + 1],
                            scalar2=s_sb[:, tg0 + j : tg0 + j + 1],
                            op0=mybir.AluOpType.is_equal,
                            op1=mybir.AluOpType.mult,
                        )
                        nc.tensor.matmul(
                            out=psum[:],
                            lhsT=msg_tiles[(pass_key, k)][:, kk, :],
                            rhs=rhs[:],
                            start=(j == 0),
                            stop=False,
                        )
                        j += 1
                # self-loops: x_own scaled by 1/count via one-hot rhs
                for gg in range(2):
                    g = 2 * p + gg
                    if g >= GROUPS:
                        continue
                    last = (gg == 1) or (g + 1 >= GROUPS)
                    xo = xopool.tile([P, P], bf16)
                    nc.sync.dma_start(
                        out=xo[:], in_=xown_dram[g * P : (g + 1) * P, :]
                    )
                    rhsS = gpool.tile([P, W2], bf16)
                    nc.vector.tensor_scalar(
                        out=rhsS[:],
                        in0=iota_sb[:],
                        scalar1=pvself_sb[:, g : g + 1],
                        scalar2=sinv_sb[:, g : g + 1],
                        op0=mybir.AluOpType.is_equal,
                        op1=mybir.AluOpType.mult,
                    )
                    nc.tensor.matmul(
                        out=psum[:],
                        lhsT=xo[:],
                        rhs=rhsS[:],
                        start=(j == 0),
                        stop=last,
                    )
                    j += 1
                agg = apool.tile([P, W2], f32)
                nc.vector.tensor_copy(out=agg[:], in_=psum[:])
                psum2 = ps2pool.tile([P, W2], f32)
                nc.tensor.matmul(
                    out=psum2[:], lhsT=wT_sb[:], rhs=agg[:], start=True, stop=True
                )
                ob = p % 2
                if ob == 0:
                    out_t = opool.tile([P, 2 * W2], f32)
                    ostart = p
                nc.scalar.activation(
                    out_t[:, ob * W2 : (ob + 1) * W2],
                    psum2[:],
                    AF.Identity,
                    bias=bias_sb[:, 0:1],
                )
                if ob == 1 or p == PAIRS - 1:
                    w = (p - ostart + 1) * W2
                    nc.sync.dma_start(
                        out=out_dram[:, ostart * W2 : ostart * W2 + w],
                        in_=out_t[:, :w],
                    )

    nc.compile()
    return nc


def _in_maps(L):
    maps = []
    for c in range(L["n_cores"]):
        maps.append(
            {
                "xA": L["xA"],
                "xB": L["xB"],
                "idxA": L["idxA_cores"][c],
                "idxB": L["idxB_cores"][c],
                "prod": L["prod_cores"][c],
                "pv": L["pv_cores"][c],
                "cself": L["cself_cores"][c],
                "pvself": L["pvself"],
                "iota": L["iota"],
                "x_own": L["x_own"][c],
                "wT": L["wT"],
                "bias_col": L["bias_col"],
            }
        )
    return maps


def _assemble(L, outs):
    N = L["N"]
    n_cores = L["n_cores"]
    LOCAL_PAD = L["LOCAL_PAD"]
    order = L["order"]
    BLK = n_cores * P
    res = np.empty((N, P), np.float32)
    ks = np.arange(LOCAL_PAD)
    for c in range(n_cores):
        oc = np.asarray(outs[c]["out"])  # [128, OUTW]
        s_rank = BLK * (ks // P) + n_cores * (ks % P) + c
        m = s_rank < N
        res[order[s_rank[m]]] = oc[:, ks[m]].T
    return res


_CACHE = {}
LAST_EXEC_NS = None


def kernel(x, edge_index, weight, bias, *, trace=False, n_cores=N_CORES):
    global LAST_EXEC_NS
    x = np.asarray(x, dtype=np.float32)
    edge_index = np.asarray(edge_index)
    weight = np.asarray(weight, dtype=np.float32)
    bias = np.asarray(bias, dtype=np.float32)

    key = hash(edge_index.tobytes()) ^ hash((x.shape, n_cores))
    if key in _CACHE:
        L, nc = _CACHE[key]
        N, SPLIT = L["N"], L["SPLIT"]
        L["xA"] = np.ascontiguousarray(x[:SPLIT]).astype(_BF16)
        L["xB"] = (
            np.ascontiguousarray(x[SPLIT:N]).astype(_BF16)
            if N - SPLIT > 0
            else L["xB"]
        )
        order = L["order"]
        BLK = L["n_cores"] * P
        ks = np.arange(L["LOCAL_PAD"])
        for c in range(L["n_cores"]):
            s_rank = BLK * (ks // P) + L["n_cores"] * (ks % P) + c
            m = s_rank < N
            xo = np.zeros((L["LOCAL_PAD"], P), _BF16)
            xo[ks[m]] = x[order[s_rank[m]]]
            L["x_own"][c] = xo
        L["wT"] = np.ascontiguousarray(weight.T)
        L["bias_col"] = bias.reshape(P, 1)
    else:
        L = _prep(x, edge_index, weight, bias, n_cores)
        nc = _build(L)
        _CACHE.clear()
        _CACHE[key] = (L, nc)

    res = run_bass_kernel_spmd(
        nc, _in_maps(L), core_ids=list(range(n_cores)), trace=trace
    )
    LAST_EXEC_NS = res.exec_time_ns
    return _assemble(L, res.results)


# revision 5
# speedup vs baseline: 2.2418x; 2.2418x over previous
"""GCNConv (message passing + linear) on 8 Trainium2 NeuronCores.

Strategy (graph/data parallel, per sharding hint):
  - Source feature table = x itself in node-id order, cast bf16, split
    into two DRAM tables (A: ids < 32768, B: rest) to satisfy the int16
    gather-index range. No on-device prestage pass.
  - Destination nodes sorted by (cntB, snake(cntA)) and dealt in blocks
    of 8*128 across the 8 cores, so each PSUM group of 128 owned dsts
    has near-uniform per-table message counts (small tile padding).
  - Each core bulk row-gathers its per-edge messages with the Q7
    dma_gather instruction (8 tiles = 1024 rows per call, the HW max)
    directly from the bf16 x tables; message tile slot k carries the
    t-th message of owned dst k.
  - Normalization s = rsqrt(deg_src*deg_dst) is applied to message
    tiles in one batched Vector-engine multiply per gather call
    (per-slot-per-tile scalars, broadcast along features); pad slots
    get s ~ 1e-15 so they contribute nothing. Degree products are
    host-provided exact integers; the rsqrt runs on device.
  - Segment-sum on the TensorEngine: scaled message tile
    [128 slot, 128 feat] (stationary) x constant identity (streaming)
    accumulated into PSUM [feat, slot] per group. Self-loops ride a
    sequential DMA of owned rows, scaled once by 1/deg (batched), and
    one extra matmul per group.
  - Final linear via W^T matmul + bias; output is [d_out, local_dst];
    host unpermutes/transposes back to [N, d_out].

The Bass program is rebuilt per distinct edge_index (layout constants
are baked into the instruction stream); all 8 cores share one program
and differ only in their input data.
"""

import numpy as np

try:
    import ml_dtypes

    _BF16 = ml_dtypes.bfloat16
except ImportError:  # pragma: no cover
    _BF16 = None

import concourse.bacc as bacc
import concourse.bass as bass
import concourse.mybir as mybir
import concourse.tile as tile
from concourse.bass_utils import run_bass_kernel_spmd
from concourse.library_config import mlp as _mlp_lib
from concourse.masks import make_identity
from concourse.tile_rust import add_dep_helper

P = 128
N_CORES = 8
TILES_PER_CALL = 8  # gather granularity; 1024 idxs = max per dma_gather call
SPLIT_MAX = 32768  # int16 gather index range per table
PAD_PROD = 1.0e30  # pad-slot degree product: rsqrt ~ 1e-15 -> contribution ~0


def _wrap_idx16(linear_idx):
    """[n] int -> [128, n/16] int16 in the 16-partition wrapped, 8x
    replicated layout dma_gather expects (slot i at [i%16, i//16])."""
    n = linear_idx.shape[0]
    assert n % 16 == 0
    w = linear_idx.reshape(-1, 16).T.astype(np.int16)  # [16, n/16]
    return np.tile(w, (8, 1))


# ----------------------------------------------------------------------------
# Host-side layout construction (sharding / index relabeling; degree
# products are exact integers — the rsqrt normalization happens on device).
# ----------------------------------------------------------------------------
def _prep(x, edge_index, weight, bias, n_cores):
    N, D = x.shape
    assert D == P
    src = np.asarray(edge_index[0], dtype=np.int64)
    dst = np.asarray(edge_index[1], dtype=np.int64)
    E = src.shape[0]

    deg = np.bincount(dst, minlength=N)
    count = (deg + 1).astype(np.int64)  # self-loop included

    SPLIT = min(SPLIT_MAX, N)
    NB_real = N - SPLIT

    in_A_src = src < SPLIT
    cntA = np.bincount(dst[in_A_src], minlength=N).astype(np.int64)
    cntB = deg - cntA

    # dst ownership: sort by cntB, snake cntA within runs, deal 8*128 blocks
    snake = np.where(cntB % 2 == 0, cntA, (1 << 20) - cntA)
    order = np.lexsort((snake, cntB))
    BLK = n_cores * P
    GROUPS = (N + BLK - 1) // BLK
    LOCAL_PAD = GROUPS * P

    cA_s = cntA[order]
    cB_s = cntB[order]
    TgA, TgB = [], []
    for g in range(GROUPS):
        lo, hi = BLK * g, min(BLK * (g + 1), N)
        TgA.append(int(cA_s[lo:hi].max()) if lo < hi else 0)
        TgB.append(int(cB_s[lo:hi].max()) if lo < hi else 0)
    toffsA = np.zeros(GROUPS + 1, np.int64)
    toffsA[1:] = np.cumsum(TgA)
    toffsB = np.zeros(GROUPS + 1, np.int64)
    toffsB[1:] = np.cumsum(TgB)
    T_totalA = int(toffsA[-1])
    T_totalB = int(toffsB[-1])

    # edges grouped per dst node id, A-sources first within each dst
    eorder = np.lexsort(((~in_A_src).astype(np.int8), dst))
    esrc = src[eorder]
    starts = np.zeros(N + 1, np.int64)
    starts[1:] = np.cumsum(deg)

    xf = np.asarray(x, dtype=np.float32)
    xA = np.ascontiguousarray(xf[:SPLIT]).astype(_BF16)
    xB = (
        np.ascontiguousarray(xf[SPLIT:N]).astype(_BF16)
        if NB_real > 0
        else np.zeros((P, P), _BF16)
    )

    idxA_cores = np.zeros((n_cores, P, 8 * max(T_totalA, 1)), np.int16)
    idxB_cores = np.zeros((n_cores, P, 8 * max(T_totalB, 1)), np.int16)
    prodA_cores = np.full((n_cores, P, max(T_totalA, 1)), PAD_PROD, np.float32)
    prodB_cores = np.full((n_cores, P, max(T_totalB, 1)), PAD_PROD, np.float32)
    cself_cores = np.ones((n_cores, P, GROUPS), np.float32)
    x_own = np.zeros((n_cores, LOCAL_PAD, P), _BF16)
    prange = np.arange(P)

    for c in range(n_cores):
        linA = np.zeros(max(T_totalA, 1) * P, np.int64)
        linB = np.zeros(max(T_totalB, 1) * P, np.int64)
        for g in range(GROUPS):
            s_rank = BLK * g + n_cores * prange + c
            valid = s_rank < N
            nd = order[np.minimum(s_rank, N - 1)]
            ca = np.where(valid, cntA[nd], 0)
            cb = np.where(valid, cntB[nd], 0)
            st = starts[nd]
            cn = count[nd]

            TA = TgA[g]
            if TA > 0:
                colsA = np.arange(TA)[None, :]
                pickA = st[:, None] + colsA
                takeA = (colsA < ca[:, None]) & valid[:, None]
                srcA = esrc[np.minimum(pickA, max(E - 1, 0))]
                base = int(toffsA[g]) * P
                # tile-major: linear pos (toffsA[g]+t)*128 + k
                linA[base : base + TA * P] = np.where(takeA, srcA, 0).T.ravel()
                prodA_cores[c][:, int(toffsA[g]) : int(toffsA[g]) + TA] = np.where(
                    takeA, count[np.minimum(srcA, N - 1)] * cn[:, None], PAD_PROD
                )

            TB = TgB[g]
            if TB > 0:
                colsB = np.arange(TB)[None, :]
                pickB = st[:, None] + ca[:, None] + colsB
                takeB = (colsB < cb[:, None]) & valid[:, None]
                srcB = esrc[np.minimum(pickB, max(E - 1, 0))] - SPLIT
                base = int(toffsB[g]) * P
                linB[base : base + TB * P] = np.where(takeB, srcB, 0).T.ravel()
                prodB_cores[c][:, int(toffsB[g]) : int(toffsB[g]) + TB] = np.where(
                    takeB,
                    count[np.minimum(srcB + SPLIT, N - 1)] * cn[:, None],
                    PAD_PROD,
                )

        assert linA.min() >= 0 and linA.max() < max(SPLIT, 1)
        idxA_cores[c] = _wrap_idx16(linA)
        if T_totalB:
            assert linB.min() >= 0 and (NB_real == 0 or linB.max() < NB_real)
            idxB_cores[c] = _wrap_idx16(linB)

        ks = np.arange(LOCAL_PAD)
        s_rank = BLK * (ks // P) + n_cores * (ks % P) + c
        m = s_rank < N
        x_own[c][ks[m]] = xf[order[s_rank[m]]]
        cs = np.ones(LOCAL_PAD, np.float32)
        cs[ks[m]] = count[order[s_rank[m]]]
        cself_cores[c] = cs.reshape(GROUPS, P).T

    wT = np.ascontiguousarray(np.asarray(weight, dtype=np.float32).T)
    bias_col = np.asarray(bias, dtype=np.float32).reshape(P, 1)

    return dict(
        N=N,
        D=D,
        E=E,
        n_cores=n_cores,
        SPLIT=SPLIT,
        NB_real=NB_real,
        GROUPS=GROUPS,
        LOCAL_PAD=LOCAL_PAD,
        TgA=TgA,
        TgB=TgB,
        toffsA=toffsA,
        toffsB=toffsB,
        T_totalA=T_totalA,
        T_totalB=T_totalB,
        xA=xA,
        xB=xB,
        x_own=x_own,
        prodA_cores=prodA_cores,
        prodB_cores=prodB_cores,
        cself_cores=cself_cores,
        idxA_cores=idxA_cores,
        idxB_cores=idxB_cores,
        wT=wT,
        bias_col=bias_col,
        order=order,
    )


# ----------------------------------------------------------------------------
# Device program
# ----------------------------------------------------------------------------
def _build(L):
    GROUPS = L["GROUPS"]
    TgA, TgB = L["TgA"], L["TgB"]
    toffsA, toffsB = L["toffsA"], L["toffsB"]
    T_totalA, T_totalB = L["T_totalA"], L["T_totalB"]
    LOCAL_PAD = L["LOCAL_PAD"]
    NAr = L["SPLIT"]
    NBr = max(L["NB_real"], P)
    f32 = mybir.dt.float32
    bf16 = mybir.dt.bfloat16
    i16 = mybir.dt.int16
    AF = mybir.ActivationFunctionType

    nc = bacc.Bacc("TRN2", debug=False, num_devices=L["n_cores"], num_swdge_queues=4)
    xA_dram = nc.dram_tensor("xA", [NAr, P], bf16, kind="ExternalInput")
    xB_dram = nc.dram_tensor("xB", [NBr, P], bf16, kind="ExternalInput")
    idxA_dram = nc.dram_tensor(
        "idxA", [P, 8 * max(T_totalA, 1)], i16, kind="ExternalInput"
    )
    idxB_dram = nc.dram_tensor(
        "idxB", [P, 8 * max(T_totalB, 1)], i16, kind="ExternalInput"
    )
    prodA_dram = nc.dram_tensor(
        "prodA", [P, max(T_totalA, 1)], f32, kind="ExternalInput"
    )
    prodB_dram = nc.dram_tensor(
        "prodB", [P, max(T_totalB, 1)], f32, kind="ExternalInput"
    )
    cself_dram = nc.dram_tensor("cself", [P, GROUPS], f32, kind="ExternalInput")
    xown_dram = nc.dram_tensor("x_own", [LOCAL_PAD, P], bf16, kind="ExternalInput")
    wT_dram = nc.dram_tensor("wT", [P, P], f32, kind="ExternalInput")
    bias_dram = nc.dram_tensor("bias_col", [P, 1], f32, kind="ExternalInput")
    out_dram = nc.dram_tensor("out", [P, LOCAL_PAD], f32, kind="ExternalOutput")

    with tile.TileContext(nc) as tc:
        with (
            tc.tile_pool(name="const", bufs=1) as cpool,
            tc.tile_pool(name="msg", bufs=20) as mpool,
            tc.tile_pool(name="agg", bufs=4) as apool,
            tc.tile_pool(name="outs", bufs=2) as opool,
            tc.tile_pool(name="ps", bufs=5, space="PSUM") as pspool,
            tc.tile_pool(name="ps2", bufs=2, space="PSUM") as ps2pool,
        ):
            lib_inst = nc.gpsimd.load_library(_mlp_lib)

            # ---- constant loads (idx first: gathers depend on them)
            idxA_sb = cpool.tile([P, 8 * max(T_totalA, 1)], i16)
            nc.sync.dma_start(out=idxA_sb[:], in_=idxA_dram[:])
            idxB_sb = cpool.tile([P, 8 * max(T_totalB, 1)], i16)
            nc.sync.dma_start(out=idxB_sb[:], in_=idxB_dram[:])
            prodA_sb = cpool.tile([P, max(T_totalA, 1)], f32)
            nc.sync.dma_start(out=prodA_sb[:], in_=prodA_dram[:])
            prodB_sb = cpool.tile([P, max(T_totalB, 1)], f32)
            nc.sync.dma_start(out=prodB_sb[:], in_=prodB_dram[:])
            cself_sb = cpool.tile([P, GROUPS], f32)
            nc.sync.dma_start(out=cself_sb[:], in_=cself_dram[:])
            wT_sb = cpool.tile([P, P], f32)
            nc.sync.dma_start(out=wT_sb[:], in_=wT_dram[:])
            bias_sb = cpool.tile([P, 1], f32)
            nc.sync.dma_start(out=bias_sb[:], in_=bias_dram[:])
            ident_sb = cpool.tile([P, P], bf16)
            make_identity(nc, ident_sb[:])

            # ---- normalization scales: s = rsqrt(count_src*count_dst)
            sA_sb = cpool.tile([P, max(T_totalA, 1)], f32)
            nc.scalar.sqrt(sA_sb[:], prodA_sb[:])
            nc.vector.reciprocal(sA_sb[:], sA_sb[:])
            sB_sb = cpool.tile([P, max(T_totalB, 1)], f32)
            nc.scalar.sqrt(sB_sb[:], prodB_sb[:])
            nc.vector.reciprocal(sB_sb[:], sB_sb[:])
            sinv_sb = cpool.tile([P, GROUPS], f32)
            nc.vector.reciprocal(sinv_sb[:], cself_sb[:])

            # ---- self-loop features, scaled once: xos = x_own * (1/count)
            xos_sb = cpool.tile([P, GROUPS, P], bf16)
            nc.sync.dma_start(
                out=xos_sb[:],
                in_=xown_dram[:, :].rearrange("(g p) f -> p g f", p=P),
            )
            nc.vector.tensor_tensor(
                out=xos_sb[:],
                in0=xos_sb[:],
                in1=sinv_sb[:].unsqueeze(2).broadcast_to([P, GROUPS, P]),
                op=mybir.AluOpType.mult,
            )

            # ---- gather calls (lazy, in consumption order), scaled per call
            msg_tiles = {}
            qrr = [0]

            def ensure_call(pass_key, k):
                key = (pass_key, k)
                if key in msg_tiles:
                    return
                T_tot = T_totalA if pass_key == "A" else T_totalB
                u_src = xA_dram if pass_key == "A" else xB_dram
                idx_sb = idxA_sb if pass_key == "A" else idxB_sb
                s_sb = sA_sb if pass_key == "A" else sB_sb
                t0 = k * TILES_PER_CALL
                cnt = min(TILES_PER_CALL, T_tot - t0)
                m = mpool.tile([P, TILES_PER_CALL, P], bf16)
                g_inst = nc.gpsimd.dma_gather(
                    m[:, :cnt, :],
                    u_src[:, :],
                    idx_sb[:, 8 * t0 : 8 * (t0 + cnt)],
                    cnt * P,
                    cnt * P,
                    P,
                    queue_num=qrr[0] % 4,
                )
                qrr[0] += 1
                add_dep_helper(g_inst.ins, lib_inst.ins, reason="ucode lib before gather")
                nc.vector.tensor_tensor(
                    out=m[:, :cnt, :],
                    in0=m[:, :cnt, :],
                    in1=s_sb[:, t0 : t0 + cnt]
                    .unsqueeze(2)
                    .broadcast_to([P, cnt, P]),
                    op=mybir.AluOpType.mult,
                )
                msg_tiles[key] = m

            # ---- per dst-group: segment-sum on PE + linear + bias
            out_t = None
            ostart = 0
            for g in range(GROUPS):
                psum = pspool.tile([P, P], f32)
                j = 0
                for pass_key, Tp, toffs in (
                    ("A", TgA[g], toffsA),
                    ("B", TgB[g], toffsB),
                ):
                    for jj in range(Tp):
                        t = int(toffs[g]) + jj
                        k, kk = divmod(t, TILES_PER_CALL)
                        ensure_call(pass_key, k)
                        nc.tensor.matmul(
                            out=psum[:],
                            lhsT=msg_tiles[(pass_key, k)][:, kk, :],
                            rhs=ident_sb[:],
                            start=(j == 0),
                            stop=False,
                        )
                        j += 1
                # self-loop
                nc.tensor.matmul(
                    out=psum[:],
                    lhsT=xos_sb[:, g, :],
                    rhs=ident_sb[:],
                    start=(j == 0),
                    stop=True,
                )
                agg = apool.tile([P, P], f32)
                nc.vector.tensor_copy(out=agg[:], in_=psum[:])
                psum2 = ps2pool.tile([P, P], f32)
                nc.tensor.matmul(
                    out=psum2[:], lhsT=wT_sb[:], rhs=agg[:], start=True, stop=True
                )
                ob = g % 4
                if ob == 0:
                    out_t = opool.tile([P, 4 * P], f32)
                    ostart = g
                nc.scalar.activation(
                    out_t[:, ob * P : (ob + 1) * P],
                    psum2[:],
                    AF.Identity,
                    bias=bias_sb[:, 0:1],
                )
                if ob == 3 or g == GROUPS - 1:
                    w = (g - ostart + 1) * P
                    nc.sync.dma_start(
                        out=out_dram[:, ostart * P : ostart * P + w],
                        in_=out_t[:, :w],
                    )

    nc.compile()
    return nc


def _in_maps(L):
    maps = []
    for c in range(L["n_cores"]):
        maps.append(
            {
                "xA": L["xA"],
                "xB": L["xB"],
                "idxA": L["idxA_cores"][c],
                "idxB": L["idxB_cores"][c],
                "prodA": L["prodA_cores"][c],
                "prodB": L["prodB_cores"][c],
                "cself": L["cself_cores"][c],
                "x_own": L["x_own"][c],
                "wT": L["wT"],
                "bias_col": L["bias_col"],
            }
        )
    return maps


def _assemble(L, outs):
    N = L["N"]
    n_cores = L["n_cores"]
    LOCAL_PAD = L["LOCAL_PAD"]
    order = L["order"]
    BLK = n_cores * P
    res = np.empty((N, P), np.float32)
    ks = np.arange(LOCAL_PAD)
    for c in range(n_cores):
        oc = np.asarray(outs[c]["out"])  # [128, LOCAL_PAD]
        s_rank = BLK * (ks // P) + n_cores * (ks % P) + c
        m = s_rank < N
        res[order[s_rank[m]]] = oc[:, ks[m]].T
    return res


_CACHE = {}
LAST_EXEC_NS = None


def kernel(x, edge_index, weight, bias, *, trace=False, n_cores=N_CORES):
    global LAST_EXEC_NS
    x = np.asarray(x, dtype=np.float32)
    edge_index = np.asarray(edge_index)
    weight = np.asarray(weight, dtype=np.float32)
    bias = np.asarray(bias, dtype=np.float32)

    key = hash(edge_index.tobytes()) ^ hash((x.shape, n_cores))
    if key in _CACHE:
        L, nc = _CACHE[key]
        N, SPLIT = L["N"], L["SPLIT"]
        L["xA"] = np.ascontiguousarray(x[:SPLIT]).astype(_BF16)
        if N - SPLIT > 0:
            L["xB"] = np.ascontiguousarray(x[SPLIT:N]).astype(_BF16)
        order = L["order"]
        BLK = L["n_cores"] * P
        ks = np.arange(L["LOCAL_PAD"])
        for c in range(L["n_cores"]):
            s_rank = BLK * (ks // P) + L["n_cores"] * (ks % P) + c
            m = s_rank < N
            xo = np.zeros((L["LOCAL_PAD"], P), _BF16)
            xo[ks[m]] = x[order[s_rank[m]]]
            L["x_own"][c] = xo
        L["wT"] = np.ascontiguousarray(weight.T)
        L["bias_col"] = bias.reshape(P, 1)
    else:
        L = _prep(x, edge_index, weight, bias, n_cores)
        nc = _build(L)
        _CACHE.clear()
        _CACHE[key] = (L, nc)

    res = run_bass_kernel_spmd(
        nc, _in_maps(L), core_ids=list(range(n_cores)), trace=trace
    )
    LAST_EXEC_NS = res.exec_time_ns
    return _assemble(L, res.results)


# revision 10
# speedup vs baseline: 2.3705x; 1.0574x over previous
"""GCNConv (message passing + linear) on 8 Trainium2 NeuronCores.

Strategy (graph/data parallel, per sharding hint):
  - Source feature table = x itself in node-id order, cast bf16, split
    into two DRAM tables (A: ids < 32768, B: rest) to satisfy the int16
    gather-index range. No on-device prestage pass.
  - Destination nodes sorted by (cntB, snake(cntA)) and dealt in blocks
    of 8*128 across the 8 cores, so each PSUM group of 128 owned dsts
    has near-uniform per-table message counts (small tile padding).
  - Each core bulk row-gathers its per-edge messages with the Q7
    dma_gather instruction (8 tiles = 1024 rows per call, the HW max)
    directly from the bf16 x tables; message tile slot k carries the
    t-th message of owned dst k.
  - Normalization s = rsqrt(deg_src*deg_dst) is applied to message
    tiles in one batched Vector-engine multiply per gather call
    (per-slot-per-tile scalars, broadcast along features); pad slots
    get s ~ 1e-15 so they contribute nothing. Degree products are
    host-provided exact integers; the rsqrt runs on device.
  - Segment-sum on the TensorEngine: scaled message tile
    [128 slot, 128 feat] (stationary) x constant identity (streaming)
    accumulated into PSUM [feat, slot] per group. Self-loops ride a
    sequential DMA of owned rows, scaled once by 1/deg (batched), and
    one extra matmul per group.
  - Final linear via W^T matmul + bias; output is [d_out, local_dst];
    host unpermutes/transposes back to [N, d_out].

The Bass program is rebuilt per distinct edge_index (layout constants
are baked into the instruction stream); all 8 cores share one program
and differ only in their input data.
"""

import numpy as np

try:
    import ml_dtypes

    _BF16 = ml_dtypes.bfloat16
except ImportError:  # pragma: no cover
    _BF16 = None

import concourse.bacc as bacc
import concourse.bass as bass
import concourse.mybir as mybir
import concourse.tile as tile
from concourse.bass_utils import run_bass_kernel_spmd
from concourse.library_config import mlp as _mlp_lib
from concourse.masks import make_identity
from concourse.tile_rust import add_dep_helper

P = 128
N_CORES = 8
TILES_PER_CALL = 8  # gather granularity; 1024 idxs = max per dma_gather call
SPLIT_MAX = 32768  # int16 gather index range per table
PAD_PROD = 1.0e30  # pad-slot degree product: rsqrt ~ 1e-15 -> contribution ~0


def _wrap_idx16(linear_idx):
    """[n] int -> [128, n/16] int16 in the 16-partition wrapped, 8x
    replicated layout dma_gather expects (slot i at [i%16, i//16])."""
    n = linear_idx.shape[0]
    assert n % 16 == 0
    w = linear_idx.reshape(-1, 16).T.astype(np.int16)  # [16, n/16]
    return np.tile(w, (8, 1))


# ----------------------------------------------------------------------------
# Host-side layout construction (sharding / index relabeling; degree
# products are exact integers — the rsqrt normalization happens on device).
# ----------------------------------------------------------------------------
def _prep(x, edge_index, weight, bias, n_cores):
    N, D = x.shape
    assert D == P
    src = np.asarray(edge_index[0], dtype=np.int64)
    dst = np.asarray(edge_index[1], dtype=np.int64)
    E = src.shape[0]

    deg = np.bincount(dst, minlength=N)
    count = (deg + 1).astype(np.int64)  # self-loop included

    SPLIT = min(SPLIT_MAX, N)
    NB_real = N - SPLIT

    in_A_src = src < SPLIT
    cntA = np.bincount(dst[in_A_src], minlength=N).astype(np.int64)
    cntB = deg - cntA

    # dst ownership: sort by cntB, snake cntA within runs, deal 8*128 blocks
    snake = np.where(cntB % 2 == 0, cntA, (1 << 20) - cntA)
    order = np.lexsort((snake, cntB))
    BLK = n_cores * P
    GROUPS = (N + BLK - 1) // BLK
    LOCAL_PAD = GROUPS * P

    cA_s = cntA[order]
    cB_s = cntB[order]
    TgA, TgB = [], []
    for g in range(GROUPS):
        lo, hi = BLK * g, min(BLK * (g + 1), N)
        TgA.append(int(cA_s[lo:hi].max()) if lo < hi else 0)
        TgB.append(int(cB_s[lo:hi].max()) if lo < hi else 0)
    toffsA = np.zeros(GROUPS + 1, np.int64)
    toffsA[1:] = np.cumsum(TgA)
    toffsB = np.zeros(GROUPS + 1, np.int64)
    toffsB[1:] = np.cumsum(TgB)
    T_totalA = int(toffsA[-1])
    T_totalB = int(toffsB[-1])

    # edges grouped per dst node id, A-sources first within each dst
    eorder = np.lexsort(((~in_A_src).astype(np.int8), dst))
    esrc = src[eorder]
    starts = np.zeros(N + 1, np.int64)
    starts[1:] = np.cumsum(deg)

    xf = np.asarray(x, dtype=np.float32)
    xA = np.ascontiguousarray(xf[:SPLIT]).astype(_BF16)
    xB = (
        np.ascontiguousarray(xf[SPLIT:N]).astype(_BF16)
        if NB_real > 0
        else np.zeros((P, P), _BF16)
    )

    idxA_cores = np.zeros((n_cores, P, 8 * max(T_totalA, 1)), np.int16)
    idxB_cores = np.zeros((n_cores, P, 8 * max(T_totalB, 1)), np.int16)
    prodA_cores = np.full((n_cores, P, max(T_totalA, 1)), PAD_PROD, np.float32)
    prodB_cores = np.full((n_cores, P, max(T_totalB, 1)), PAD_PROD, np.float32)
    cself_cores = np.ones((n_cores, P, GROUPS), np.float32)
    # partition-major self-features: row p holds group-concatenated x rows of
    # the dsts at slot p, so the SBUF load is one contiguous stripe/partition
    x_own = np.zeros((n_cores, P, GROUPS * P), _BF16)
    prange = np.arange(P)

    for c in range(n_cores):
        linA = np.zeros(max(T_totalA, 1) * P, np.int64)
        linB = np.zeros(max(T_totalB, 1) * P, np.int64)
        for g in range(GROUPS):
            s_rank = BLK * g + n_cores * prange + c
            valid = s_rank < N
            nd = order[np.minimum(s_rank, N - 1)]
            ca = np.where(valid, cntA[nd], 0)
            cb = np.where(valid, cntB[nd], 0)
            st = starts[nd]
            cn = count[nd]

            TA = TgA[g]
            if TA > 0:
                colsA = np.arange(TA)[None, :]
                pickA = st[:, None] + colsA
                takeA = (colsA < ca[:, None]) & valid[:, None]
                srcA = esrc[np.minimum(pickA, max(E - 1, 0))]
                base = int(toffsA[g]) * P
                # tile-major: linear pos (toffsA[g]+t)*128 + k
                linA[base : base + TA * P] = np.where(takeA, srcA, 0).T.ravel()
                prodA_cores[c][:, int(toffsA[g]) : int(toffsA[g]) + TA] = np.where(
                    takeA, count[np.minimum(srcA, N - 1)] * cn[:, None], PAD_PROD
                )

            TB = TgB[g]
            if TB > 0:
                colsB = np.arange(TB)[None, :]
                pickB = st[:, None] + ca[:, None] + colsB
                takeB = (colsB < cb[:, None]) & valid[:, None]
                srcB = esrc[np.minimum(pickB, max(E - 1, 0))] - SPLIT
                base = int(toffsB[g]) * P
                linB[base : base + TB * P] = np.where(takeB, srcB, 0).T.ravel()
                prodB_cores[c][:, int(toffsB[g]) : int(toffsB[g]) + TB] = np.where(
                    takeB,
                    count[np.minimum(srcB + SPLIT, N - 1)] * cn[:, None],
                    PAD_PROD,
                )

        assert linA.min() >= 0 and linA.max() < max(SPLIT, 1)
        idxA_cores[c] = _wrap_idx16(linA)
        if T_totalB:
            assert linB.min() >= 0 and (NB_real == 0 or linB.max() < NB_real)
            idxB_cores[c] = _wrap_idx16(linB)

        ks = np.arange(LOCAL_PAD)
        s_rank = BLK * (ks // P) + n_cores * (ks % P) + c
        m = s_rank < N
        xo = np.zeros((GROUPS, P, P), np.float32)  # [g, slot, feat]
        xo.reshape(LOCAL_PAD, P)[ks[m]] = xf[order[s_rank[m]]]
        x_own[c] = xo.transpose(1, 0, 2).reshape(P, GROUPS * P)
        cs = np.ones(LOCAL_PAD, np.float32)
        cs[ks[m]] = count[order[s_rank[m]]]
        cself_cores[c] = cs.reshape(GROUPS, P).T

    wT = np.ascontiguousarray(np.asarray(weight, dtype=np.float32).T)
    bias_col = np.asarray(bias, dtype=np.float32).reshape(P, 1)

    return dict(
        N=N,
        D=D,
        E=E,
        n_cores=n_cores,
        SPLIT=SPLIT,
        NB_real=NB_real,
        GROUPS=GROUPS,
        LOCAL_PAD=LOCAL_PAD,
        TgA=TgA,
        TgB=TgB,
        toffsA=toffsA,
        toffsB=toffsB,
        T_totalA=T_totalA,
        T_totalB=T_totalB,
        xA=xA,
        xB=xB,
        x_own=x_own,
        prodA_cores=prodA_cores,
        prodB_cores=prodB_cores,
        cself_cores=cself_cores,
        idxA_cores=idxA_cores,
        idxB_cores=idxB_cores,
        wT=wT,
        bias_col=bias_col,
        order=order,
    )


# ----------------------------------------------------------------------------
# Device program
# ----------------------------------------------------------------------------
def _build(L):
    GROUPS = L["GROUPS"]
    TgA, TgB = L["TgA"], L["TgB"]
    toffsA, toffsB = L["toffsA"], L["toffsB"]
    T_totalA, T_totalB = L["T_totalA"], L["T_totalB"]
    LOCAL_PAD = L["LOCAL_PAD"]
    NAr = L["SPLIT"]
    NBr = max(L["NB_real"], P)
    f32 = mybir.dt.float32
    bf16 = mybir.dt.bfloat16
    i16 = mybir.dt.int16
    AF = mybir.ActivationFunctionType

    nc = bacc.Bacc("TRN2", debug=False, num_devices=L["n_cores"], num_swdge_queues=4)
    xA_dram = nc.dram_tensor("xA", [NAr, P], bf16, kind="ExternalInput")
    xB_dram = nc.dram_tensor("xB", [NBr, P], bf16, kind="ExternalInput")
    idxA_dram = nc.dram_tensor(
        "idxA", [P, 8 * max(T_totalA, 1)], i16, kind="ExternalInput"
    )
    idxB_dram = nc.dram_tensor(
        "idxB", [P, 8 * max(T_totalB, 1)], i16, kind="ExternalInput"
    )
    prodA_dram = nc.dram_tensor(
        "prodA", [P, max(T_totalA, 1)], f32, kind="ExternalInput"
    )
    prodB_dram = nc.dram_tensor(
        "prodB", [P, max(T_totalB, 1)], f32, kind="ExternalInput"
    )
    cself_dram = nc.dram_tensor("cself", [P, GROUPS], f32, kind="ExternalInput")
    xown_dram = nc.dram_tensor("x_own", [P, GROUPS * P], bf16, kind="ExternalInput")
    wT_dram = nc.dram_tensor("wT", [P, P], f32, kind="ExternalInput")
    bias_dram = nc.dram_tensor("bias_col", [P, 1], f32, kind="ExternalInput")
    out_dram = nc.dram_tensor("out", [P, LOCAL_PAD], f32, kind="ExternalOutput")

    with tile.TileContext(nc) as tc:
        with (
            tc.tile_pool(name="const", bufs=1) as cpool,
            tc.tile_pool(name="msg", bufs=20) as mpool,
            tc.tile_pool(name="agg", bufs=4) as apool,
            tc.tile_pool(name="outs", bufs=2) as opool,
            tc.tile_pool(name="ps", bufs=5, space="PSUM") as pspool,
            tc.tile_pool(name="ps2", bufs=2, space="PSUM") as ps2pool,
        ):
            lib_inst = nc.gpsimd.load_library(_mlp_lib)

            # ---- constant loads (idx first: gathers depend on them)
            idxA_sb = cpool.tile([P, 8 * max(T_totalA, 1)], i16)
            nc.sync.dma_start(out=idxA_sb[:], in_=idxA_dram[:])
            idxB_sb = cpool.tile([P, 8 * max(T_totalB, 1)], i16)
            nc.sync.dma_start(out=idxB_sb[:], in_=idxB_dram[:])
            prodA_sb = cpool.tile([P, max(T_totalA, 1)], f32)
            nc.sync.dma_start(out=prodA_sb[:], in_=prodA_dram[:])
            prodB_sb = cpool.tile([P, max(T_totalB, 1)], f32)
            nc.sync.dma_start(out=prodB_sb[:], in_=prodB_dram[:])
            cself_sb = cpool.tile([P, GROUPS], f32)
            nc.sync.dma_start(out=cself_sb[:], in_=cself_dram[:])
            wT_sb = cpool.tile([P, P], f32)
            nc.sync.dma_start(out=wT_sb[:], in_=wT_dram[:])
            bias_sb = cpool.tile([P, 1], f32)
            nc.sync.dma_start(out=bias_sb[:], in_=bias_dram[:])
            ident_sb = cpool.tile([P, P], bf16)
            make_identity(nc, ident_sb[:])

            # ---- normalization scales: s = rsqrt(count_src*count_dst)
            sA_sb = cpool.tile([P, max(T_totalA, 1)], f32)
            nc.scalar.sqrt(sA_sb[:], prodA_sb[:])
            nc.vector.reciprocal(sA_sb[:], sA_sb[:])
            sB_sb = cpool.tile([P, max(T_totalB, 1)], f32)
            nc.scalar.sqrt(sB_sb[:], prodB_sb[:])
            nc.vector.reciprocal(sB_sb[:], sB_sb[:])
            sinv_sb = cpool.tile([P, GROUPS], f32)
            nc.vector.reciprocal(sinv_sb[:], cself_sb[:])

            # ---- self-loop features, scaled once: xos = x_own * (1/count)
            xos_sb = cpool.tile([P, GROUPS, P], bf16)
            nc.sync.dma_start(
                out=xos_sb[:],
                in_=xown_dram[:, :].rearrange("p (g f) -> p g f", f=P),
            )
            nc.vector.tensor_tensor(
                out=xos_sb[:],
                in0=xos_sb[:],
                in1=sinv_sb[:].unsqueeze(2).broadcast_to([P, GROUPS, P]),
                op=mybir.AluOpType.mult,
            )

            # ---- gather calls (lazy, in consumption order), scaled per call
            msg_tiles = {}
            qrr = [0]

            def ensure_call(pass_key, k):
                key = (pass_key, k)
                if key in msg_tiles:
                    return
                T_tot = T_totalA if pass_key == "A" else T_totalB
                u_src = xA_dram if pass_key == "A" else xB_dram
                idx_sb = idxA_sb if pass_key == "A" else idxB_sb
                s_sb = sA_sb if pass_key == "A" else sB_sb
                t0 = k * TILES_PER_CALL
                cnt = min(TILES_PER_CALL, T_tot - t0)
                m = mpool.tile([P, TILES_PER_CALL, P], bf16)
                g_inst = nc.gpsimd.dma_gather(
                    m[:, :cnt, :],
                    u_src[:, :],
                    idx_sb[:, 8 * t0 : 8 * (t0 + cnt)],
                    cnt * P,
                    cnt * P,
                    P,
                    queue_num=qrr[0] % 4,
                )
                qrr[0] += 1
                add_dep_helper(g_inst.ins, lib_inst.ins, reason="ucode lib before gather")
                nc.vector.tensor_tensor(
                    out=m[:, :cnt, :],
                    in0=m[:, :cnt, :],
                    in1=s_sb[:, t0 : t0 + cnt]
                    .unsqueeze(2)
                    .broadcast_to([P, cnt, P]),
                    op=mybir.AluOpType.mult,
                )
                msg_tiles[key] = m

            # ---- per dst-group: segment-sum on PE + linear + bias
            out_t = None
            ostart = 0
            for g in range(GROUPS):
                psum = pspool.tile([P, P], f32)
                j = 0
                for pass_key, Tp, toffs in (
                    ("A", TgA[g], toffsA),
                    ("B", TgB[g], toffsB),
                ):
                    for jj in range(Tp):
                        t = int(toffs[g]) + jj
                        k, kk = divmod(t, TILES_PER_CALL)
                        ensure_call(pass_key, k)
                        nc.tensor.matmul(
                            out=psum[:],
                            lhsT=msg_tiles[(pass_key, k)][:, kk, :],
                            rhs=ident_sb[:],
                            start=(j == 0),
                            stop=False,
                        )
                        j += 1
                # self-loop
                nc.tensor.matmul(
                    out=psum[:],
                    lhsT=xos_sb[:, g, :],
                    rhs=ident_sb[:],
                    start=(j == 0),
                    stop=True,
                )
                agg = apool.tile([P, P], f32)
                nc.vector.tensor_copy(out=agg[:], in_=psum[:])
                psum2 = ps2pool.tile([P, P], f32)
                nc.tensor.matmul(
                    out=psum2[:], lhsT=wT_sb[:], rhs=agg[:], start=True, stop=True
                )
                ob = g % 4
                if ob == 0:
                    out_t = opool.tile([P, 4 * P], f32)
                    ostart = g
                nc.scalar.activation(
                    out_t[:, ob * P : (ob + 1) * P],
                    psum2[:],
                    AF.Identity,
                    bias=bias_sb[:, 0:1],
                )
                if ob == 3 or g == GROUPS - 1:
                    w = (g - ostart + 1) * P
                    nc.sync.dma_start(
                        out=out_dram[:, ostart * P : ostart * P + w],
                        in_=out_t[:, :w],
                    )

    nc.compile()
    return nc


def _in_maps(L):
    maps = []
    for c in range(L["n_cores"]):
        maps.append(
            {
                "xA": L["xA"],
                "xB": L["xB"],
                "idxA": L["idxA_cores"][c],
                "idxB": L["idxB_cores"][c],
                "prodA": L["prodA_cores"][c],
                "prodB": L["prodB_cores"][c],
                "cself": L["cself_cores"][c],
                "x_own": L["x_own"][c],
                "wT": L["wT"],
                "bias_col": L["bias_col"],
            }
        )
    return maps


def _assemble(L, outs):
    N = L["N"]
    n_cores = L["n_cores"]
    LOCAL_PAD = L["LOCAL_PAD"]
    order = L["order"]
    BLK = n_cores * P
    res = np.empty((N, P), np.float32)
    ks = np.arange(LOCAL_PAD)
    for c in range(n_cores):
        oc = np.asarray(outs[c]["out"])  # [128, LOCAL_PAD]
        s_rank = BLK * (ks // P) + n_cores * (ks % P) + c
        m = s_rank < N
        res[order[s_rank[m]]] = oc[:, ks[m]].T
    return res


_CACHE = {}
LAST_EXEC_NS = None


def kernel(x, edge_index, weight, bias, *, trace=False, n_cores=N_CORES):
    global LAST_EXEC_NS
    x = np.asarray(x, dtype=np.float32)
    edge_index = np.asarray(edge_index)
    weight = np.asarray(weight, dtype=np.float32)
    bias = np.asarray(bias, dtype=np.float32)

    key = hash(edge_index.tobytes()) ^ hash((x.shape, n_cores))
    if key in _CACHE:
        L, nc = _CACHE[key]
        N, SPLIT = L["N"], L["SPLIT"]
        L["xA"] = np.ascontiguousarray(x[:SPLIT]).astype(_BF16)
        if N - SPLIT > 0:
            L["xB"] = np.ascontiguousarray(x[SPLIT:N]).astype(_BF16)
        order = L["order"]
        BLK = L["n_cores"] * P
        GROUPS = L["GROUPS"]
        ks = np.arange(L["LOCAL_PAD"])
        for c in range(L["n_cores"]):
            s_rank = BLK * (ks // P) + L["n_cores"] * (ks % P) + c
            m = s_rank < N
            xo = np.zeros((GROUPS, P, P), np.float32)
            xo.reshape(L["LOCAL_PAD"], P)[ks[m]] = x[order[s_rank[m]]]
            L["x_own"][c] = xo.transpose(1, 0, 2).reshape(P, GROUPS * P)
        L["wT"] = np.ascontiguousarray(weight.T)
        L["bias_col"] = bias.reshape(P, 1)
    else:
        L = _prep(x, edge_index, weight, bias, n_cores)
        nc = _build(L)
        _CACHE.clear()
        _CACHE[key] = (L, nc)

    res = run_bass_kernel_spmd(
        nc, _in_maps(L), core_ids=list(range(n_cores)), trace=trace
    )
    LAST_EXEC_NS = res.exec_time_ns
    return _assemble(L, res.results)
